# revision 9
# baseline (speedup 1.0000x reference)
"""Trainium2 Bass kernel for nn_Mesh_Renderer: silhouette rasterizer.

Strategy: data-parallel over batch. Core b renders batch b's 64x64 silhouette
from 1280 triangles. Host-side work is layout only: slice per batch, gather
vertices[faces] (pure indexing, no arithmetic), transpose. All math (camera
transform, perspective divide, edge functions, coverage test, reduction) runs
on device.

Device pipeline per core:
  1. camera basis R from eye (look_at, mirrored op-for-op from the reference)
  2. v_cam = (verts - eye) @ R^T via PE matmuls  (verts pre-gathered per
     face-corner: 1280 faces x 4 corners (a,b,c,a) = 5120 columns)
  3. perspective: x_ndc = x / (z*tan + eps)
  4. edge coefficients per face-edge: e(x,y) = A*x + B*y + C
  5. rasterize 10 face-tiles of 128 faces x 4096 pixels: e-planes via
     broadcast adds, min3/max3 sign tests, visibility fold, bf16 indicator
     accumulation
  6. count = ones^T @ acc (PE partition reduction), silhouette = count > 0
"""

import sys

if "/opt/trn_rl_repo" not in sys.path:
    sys.path.insert(0, "/opt/trn_rl_repo")

import numpy as np

import concourse.bass as bass
import concourse.bacc as bacc
import concourse.tile as tile
from concourse import mybir
from concourse.bass_utils import run_bass_kernel_spmd

F32 = mybir.dt.float32
BF16 = mybir.dt.bfloat16
I32 = mybir.dt.int32
OP = mybir.AluOpType
AF = mybir.ActivationFunctionType

B, V, NF, IMG = 8, 642, 1280, 64
NPIX = IMG * IMG          # 4096
NTILE = NF // 128         # 10 face tiles
NCOL = NF * 4             # 5120 gathered corners (a, b, c, a)
EPS = 1e-8
# tan(deg2rad(15)) in float32, matching jnp.tan(jnp.deg2rad(float32(15)))
TAN_T = float(np.tan(np.deg2rad(np.float32(15.0)).astype(np.float32)))
BIG = 1e30


def _normalize3(nc, pool, v, name):
    """v [1,3] f32 -> v / (||v|| + 1e-8), mirroring the reference formula."""
    sq = pool.tile([1, 3], F32, tag=f"{name}_sq")
    nc.vector.tensor_tensor(sq[:], v[:], v[:], OP.mult)
    s = pool.tile([1, 1], F32, tag=f"{name}_s")
    nc.vector.tensor_reduce(s[:], sq[:], mybir.AxisListType.X, OP.add)
    n = pool.tile([1, 1], F32, tag=f"{name}_n")
    nc.scalar.activation(n[:], s[:], AF.Sqrt)
    # Newton refine sqrt: n1 = 0.5*(n + s/n)
    rn = pool.tile([1, 1], F32, tag=f"{name}_rn")
    nc.vector.reciprocal(rn[:], n[:])
    t = pool.tile([1, 1], F32, tag=f"{name}_t")
    nc.vector.tensor_tensor(t[:], s[:], rn[:], OP.mult)
    t2 = pool.tile([1, 1], F32, tag=f"{name}_t2")
    nc.vector.tensor_tensor(t2[:], n[:], t[:], OP.add)
    n1 = pool.tile([1, 1], F32, tag=f"{name}_n1")
    nc.vector.tensor_scalar(n1[:], t2[:], 0.5, None, OP.mult)
    d = pool.tile([1, 1], F32, tag=f"{name}_d")
    nc.vector.tensor_scalar(d[:], n1[:], EPS, None, OP.add)
    r = pool.tile([1, 1], F32, tag=f"{name}_r")
    nc.vector.reciprocal(r[:], d[:])
    # Newton refine recip: r1 = r*(2 - d*r)
    u = pool.tile([1, 1], F32, tag=f"{name}_u")
    nc.vector.tensor_tensor(u[:], d[:], r[:], OP.mult)
    u2 = pool.tile([1, 1], F32, tag=f"{name}_u2")
    nc.vector.tensor_scalar(u2[:], u[:], -1.0, 2.0, OP.mult, OP.add)
    r1 = pool.tile([1, 1], F32, tag=f"{name}_r1")
    nc.vector.tensor_tensor(r1[:], r[:], u2[:], OP.mult)
    out = pool.tile([1, 3], F32, tag=f"{name}_out")
    nc.vector.tensor_scalar(out[:], v[:], r1[:], None, OP.mult)
    return out


def _cross3(nc, pool, a, b, name):
    """cross(a, b) for [1,3] tiles via duplicated [1,6] buffers."""
    a2 = pool.tile([1, 6], F32, tag=f"{name}_a2")
    nc.vector.tensor_copy(a2[:, 0:3], a[:])
    nc.vector.tensor_copy(a2[:, 3:6], a[:])
    b2 = pool.tile([1, 6], F32, tag=f"{name}_b2")
    nc.vector.tensor_copy(b2[:, 0:3], b[:])
    nc.vector.tensor_copy(b2[:, 3:6], b[:])
    m1 = pool.tile([1, 3], F32, tag=f"{name}_m1")
    nc.vector.tensor_tensor(m1[:], a2[:, 1:4], b2[:, 2:5], OP.mult)
    m2 = pool.tile([1, 3], F32, tag=f"{name}_m2")
    nc.vector.tensor_tensor(m2[:], a2[:, 2:5], b2[:, 1:4], OP.mult)
    out = pool.tile([1, 3], F32, tag=f"{name}_out")
    nc.vector.tensor_tensor(out[:], m1[:], m2[:], OP.subtract)
    return out


def build_kernel(ctx, tc):
    nc = tc.nc
    vgt_d = nc.dram_tensor("vgt", [3, NCOL], F32, kind="ExternalInput")
    eye_d = nc.dram_tensor("eye", [3], F32, kind="ExternalInput")
    sil_d = nc.dram_tensor("sil", [NPIX], F32, kind="ExternalOutput")

    cpool = ctx.enter_context(tc.tile_pool(name="cam", bufs=1))
    ppool = ctx.enter_context(tc.tile_pool(name="proj", bufs=1))
    gpool = ctx.enter_context(tc.tile_pool(name="grid", bufs=1))

    # ---- camera basis (partition 0, tiny tiles) ----
    eyeR = cpool.tile([1, 3], F32)
    nc.sync.dma_start(eyeR[:], eye_d.ap())
    eT = cpool.tile([3, 1], F32)
    nc.sync.dma_start(eT[:], eye_d.ap())

    nege = cpool.tile([1, 3], F32)
    nc.vector.tensor_scalar(nege[:], eyeR[:], -1.0, None, OP.mult)
    z_ax = _normalize3(nc, cpool, nege, "nz")

    xr = cpool.tile([1, 3], F32)
    nc.vector.memset(xr[:], 0.0)
    nc.vector.tensor_copy(xr[:, 0:1], z_ax[:, 2:3])
    nc.vector.tensor_scalar(xr[:, 2:3], z_ax[:, 0:1], -1.0, None, OP.mult)
    x_ax = _normalize3(nc, cpool, xr, "nx")

    yr = _cross3(nc, cpool, z_ax, x_ax, "cy")
    y_ax = _normalize3(nc, cpool, yr, "ny")

    # RT[c, d] = R[d, c]; column d of RT = axis row d
    rt = cpool.tile([3, 3], F32)
    for d, axis in enumerate([x_ax, y_ax, z_ax]):
        nc.sync.dma_start(rt[:, d : d + 1], axis[:])

    # ---- projection of 5120 gathered corners ----
    vca = ppool.tile([128, 120], F32)  # [p, (chunk c, coord d)]
    with tc.tile_pool(name="vg", bufs=1) as vgp, \
         tc.tile_pool(name="pvc", bufs=1, space="PSUM") as psvc:
        vgt = vgp.tile([3, NCOL], F32)
        nc.sync.dma_start(vgt[:], vgt_d.ap())
        vme = vgp.tile([3, NCOL], F32)
        nc.vector.tensor_scalar(vme[:], vgt[:], eT[:], None, OP.subtract)
        vcp = psvc.tile([128, 120], F32)
        for c in range(40):
            nc.tensor.matmul(
                vcp[:, 3 * c : 3 * c + 3],
                vme[:, 128 * c : 128 * (c + 1)],
                rt[:],
                start=True,
                stop=True,
            )
        nc.vector.tensor_copy(vca[:], vcp[:])

    vcav = vca[:].rearrange("p (c d) -> p c d", d=3)
    vx, vy, vz = vcav[:, :, 0], vcav[:, :, 1], vcav[:, :, 2]

    dn = ppool.tile([128, 40], F32)
    nc.vector.tensor_scalar(dn[:], vz, TAN_T, EPS, OP.mult, OP.add)
    rc0 = ppool.tile([128, 40], F32)
    nc.vector.reciprocal(rc0[:], dn[:])
    t = ppool.tile([128, 40], F32)
    nc.vector.tensor_tensor(t[:], dn[:], rc0[:], OP.mult)
    t2 = ppool.tile([128, 40], F32)
    nc.vector.tensor_scalar(t2[:], t[:], -1.0, 2.0, OP.mult, OP.add)
    rc = ppool.tile([128, 40], F32)
    nc.vector.tensor_tensor(rc[:], rc0[:], t2[:], OP.mult)

    xn = ppool.tile([128, 40], F32)
    nc.vector.tensor_tensor(xn[:], vx, rc[:], OP.mult)
    yn = ppool.tile([128, 40], F32)
    nc.vector.tensor_tensor(yn[:], vy, rc[:], OP.mult)

    # ---- edge coefficients: e = A*x + B*y + C per (face, edge) ----
    xnv = xn[:].rearrange("p (ft k) -> p ft k", k=4)
    ynv = yn[:].rearrange("p (ft k) -> p ft k", k=4)
    A = ppool.tile([128, 30], F32)
    nc.vector.tensor_tensor(A[:].rearrange("p (ft k) -> p ft k", k=3),
                            ynv[:, :, 0:3], ynv[:, :, 1:4], OP.subtract)
    Bc = ppool.tile([128, 30], F32)
    nc.vector.tensor_tensor(Bc[:].rearrange("p (ft k) -> p ft k", k=3),
                            xnv[:, :, 1:4], xnv[:, :, 0:3], OP.subtract)
    p1 = ppool.tile([128, 30], F32)
    nc.vector.tensor_tensor(p1[:].rearrange("p (ft k) -> p ft k", k=3),
                            xnv[:, :, 0:3], ynv[:, :, 1:4], OP.mult)
    p2 = ppool.tile([128, 30], F32)
    nc.vector.tensor_tensor(p2[:].rearrange("p (ft k) -> p ft k", k=3),
                            ynv[:, :, 0:3], xnv[:, :, 1:4], OP.mult)
    Cc = ppool.tile([128, 30], F32)
    nc.vector.tensor_tensor(Cc[:], p1[:], p2[:], OP.subtract)

    # visibility: all three corner z > 0 (strict), as a 0/1 per-face scalar
    vz4 = vca[:].rearrange("p (ft k d) -> p ft k d", k=4, d=3)
    mz1 = ppool.tile([128, 10], F32)
    nc.vector.tensor_tensor(mz1[:], vz4[:, :, 0, 2], vz4[:, :, 1, 2], OP.min)
    mz = ppool.tile([128, 10], F32)
    nc.vector.tensor_tensor(mz[:], mz1[:], vz4[:, :, 2, 2], OP.min)
    vg = ppool.tile([128, 10], F32)
    nc.vector.tensor_scalar(vg[:], mz[:], 0.0, None, OP.is_gt)

    # ---- pixel grids ----
    it32 = gpool.tile([128, IMG], I32)
    nc.gpsimd.iota(it32[:], pattern=[[1, IMG]], base=0, channel_multiplier=0)
    itf = gpool.tile([128, IMG], F32)
    nc.vector.tensor_copy(itf[:], it32[:])
    xg = gpool.tile([128, IMG], F32)  # x_j = j/32 - 63/64 (exact)
    nc.vector.tensor_scalar(xg[:], itf[:], 1.0 / 32.0, -63.0 / 64.0, OP.mult, OP.add)
    yg = gpool.tile([128, IMG], F32)  # y_i = -x_i
    nc.vector.tensor_scalar(yg[:], xg[:], -1.0, None, OP.mult)
    ones_bf = gpool.tile([128, 1], BF16)
    nc.vector.memset(ones_bf[:], 1.0)

    # visibility weights for the counting matmuls: +vg and -vg in bf16
    vgp_bf = ppool.tile([128, 10], BF16)
    nc.vector.tensor_copy(vgp_bf[:], vg[:])
    vgn_bf = ppool.tile([128, 10], BF16)
    nc.vector.tensor_scalar(vgn_bf[:], vg[:], -1.0, None, OP.mult)

    # visible-face count V and threshold -2V (counting identity:
    # count = T/2 + V with T = sum vg*(sign(min3) - sign(max3)),
    # silhouette <=> count > 0 <=> T > -2V; all quantities are exact ints)
    vgs = ppool.tile([128, 1], F32)
    nc.vector.tensor_reduce(vgs[:], vg[:], mybir.AxisListType.X, OP.add)
    vgs_bf = ppool.tile([128, 1], BF16)
    nc.vector.tensor_copy(vgs_bf[:], vgs[:])

    # ---- rasterization ----
    # Per face-tile: e-planes for 3 edges in one fused broadcast add (DVE,
    # f32), sign via ScalarE (ACT Sign -> bf16), min/max chains in bf16 (2x
    # DVE), per-face +/-vis-weighted counting on the PE into PSUM.
    HALF = NPIX // 2            # 2048 pixels per half (i in [0,32) / [32,64))
    upool = ctx.enter_context(tc.tile_pool(name="uw", bufs=4))
    epool = ctx.enter_context(tc.tile_pool(name="e3", bufs=2))
    spool = ctx.enter_context(tc.tile_pool(name="s3", bufs=2))
    mpool = ctx.enter_context(tc.tile_pool(name="mm", bufs=6))
    pscnt = ctx.enter_context(tc.tile_pool(name="pcnt", bufs=1, space="PSUM"))
    cnt = pscnt.tile([1, NPIX], F32, tag="cnt")
    region_n = [0] * (NPIX // 512)
    TOTAL_MM = NTILE * 2 * 4  # matmuls per 512-region over the whole loop

    def count_mm(lhsT, plane, pix_off):
        # accumulate lhsT^T @ plane into cnt[pix_off : pix_off+2048]
        for c in range(4):
            off = pix_off + 512 * c
            r = off // 512
            nc.tensor.matmul(
                cnt[:, off : off + 512], lhsT[:], plane[:, 512 * c : 512 * (c + 1)],
                start=(region_n[r] == 0), stop=(region_n[r] == TOTAL_MM // 4 - 1),
            )
            region_n[r] += 1

    for ft in range(NTILE):
        u3 = upool.tile([128, 3 * IMG], F32, tag="u3")
        w3 = upool.tile([128, 3 * IMG], F32, tag="w3")
        for k in range(3):
            col = 3 * ft + k
            nc.vector.tensor_scalar(u3[:, IMG * k : IMG * (k + 1)], xg[:],
                                    A[:, col : col + 1], Cc[:, col : col + 1],
                                    OP.mult, OP.add)
            nc.vector.tensor_scalar(w3[:, IMG * k : IMG * (k + 1)], yg[:],
                                    Bc[:, col : col + 1], None, OP.mult)
        u3r = u3[:].rearrange("p (k j) -> p k j", k=3)
        w3r = w3[:].rearrange("p (k i) -> p k i", k=3)
        for h in range(2):
            e3 = epool.tile([128, 3 * HALF], F32, tag="e3")
            nc.vector.tensor_tensor(
                e3[:].rearrange("p (k i j) -> p k i j", k=3, i=IMG // 2),
                w3r[:, :, 32 * h : 32 * (h + 1)].unsqueeze(3)
                    .broadcast_to([128, 3, 32, IMG]),
                u3r.unsqueeze(2).broadcast_to([128, 3, 32, IMG]),
                OP.add,
            )
            s3 = spool.tile([128, 3 * HALF], BF16, tag="s3")
            nc.scalar.activation(s3[:], e3[:], AF.Sign)
            s3r = s3[:].rearrange("p (k x) -> p k x", k=3)
            sm1 = mpool.tile([128, HALF], BF16, tag="mm")
            nc.vector.tensor_tensor(sm1[:], s3r[:, 0], s3r[:, 1], OP.min)
            smin = mpool.tile([128, HALF], BF16, tag="mm")
            nc.vector.tensor_tensor(smin[:], sm1[:], s3r[:, 2], OP.min)
            sM1 = mpool.tile([128, HALF], BF16, tag="mm")
            nc.vector.tensor_tensor(sM1[:], s3r[:, 0], s3r[:, 1], OP.max)
            smax = mpool.tile([128, HALF], BF16, tag="mm")
            nc.vector.tensor_tensor(smax[:], sM1[:], s3r[:, 2], OP.max)
            count_mm(vgp_bf[:, ft : ft + 1], smin, HALF * h)
            count_mm(vgn_bf[:, ft : ft + 1], smax, HALF * h)

    # ---- V total, threshold, output ----
    psv = ctx.enter_context(tc.tile_pool(name="pv", bufs=1, space="PSUM"))
    vtot = psv.tile([1, 1], F32)
    nc.tensor.matmul(vtot[:], ones_bf[:], vgs_bf[:], start=True, stop=True)
    thr = gpool.tile([1, 1], F32)
    nc.vector.tensor_scalar(thr[:], vtot[:], -2.0, None, OP.mult)
    silb = gpool.tile([1, NPIX], F32)
    nc.vector.tensor_tensor(silb[:], cnt[:], thr[:].broadcast_to([1, NPIX]),
                            OP.is_gt)
    nc.sync.dma_start(sil_d.ap(), silb[:])


_NC = None


def _get_program():
    global _NC
    if _NC is None:
        nc = bacc.Bacc(
            "TRN2",
            target_bir_lowering=False,
            debug=False,
            enable_asserts=False,
            num_devices=B,
        )
        from contextlib import ExitStack

        with tile.TileContext(nc) as tc:
            with ExitStack() as ctx:
                build_kernel(ctx, tc)
        nc.compile()
        _NC = nc
    return _NC


def _host_layout(vertices, faces):
    """Pure indexing: gather per-face-corner vertices, layout [3, 5120] where
    column n = ft*512 + k*128 + p holds corner k of face ft*128+p."""
    faces4 = np.concatenate([faces, faces[:, :1]], axis=1)  # [1280, 4]
    vidx = faces4.reshape(NTILE, 128, 4).transpose(0, 2, 1).reshape(-1)  # [5120]
    out = []
    for b in range(B):
        vg = vertices[b][vidx]  # [5120, 3]
        out.append(np.ascontiguousarray(vg.T.astype(np.float32)))
    return out


def kernel(vertices, viewpoints, faces, img_size):
    vertices = np.asarray(vertices, dtype=np.float32)
    viewpoints = np.asarray(viewpoints, dtype=np.float32)
    faces = np.asarray(faces, dtype=np.int32)
    assert int(img_size) == IMG and vertices.shape == (B, V, 3)

    nc = _get_program()
    vgts = _host_layout(vertices, faces)
    in_maps = [
        {"vgt": vgts[b], "eye": np.ascontiguousarray(viewpoints[b])}
        for b in range(B)
    ]
    res = run_bass_kernel_spmd(nc, in_maps, core_ids=list(range(B)))
    sil = np.stack([res.results[b]["sil"] for b in range(B)])  # [8, 4096]
    return sil.reshape(B, 1, IMG, IMG).astype(np.float32)


if __name__ == "__main__":
    # quick self-exercise with random data
    rng = np.random.default_rng(0)
    verts = rng.standard_normal((B, V, 3), dtype=np.float32) * 0.5
    vps = rng.standard_normal((B, 3), dtype=np.float32)
    fcs = rng.integers(0, V, (NF, 3), dtype=np.int32)
    out = kernel(verts, vps, fcs, IMG)
    print(out.shape, out.sum())


# revision 11
# speedup vs baseline: 1.6946x; 1.6946x over previous
"""Trainium2 Bass kernel for nn_Mesh_Renderer: silhouette rasterizer.

Strategy: data-parallel over batch. Core b renders batch b's 64x64 silhouette
from 1280 triangles. Host-side work is layout only: slice per batch, gather
vertices[faces] (pure indexing, no arithmetic), transpose. All math (camera
transform, perspective divide, edge functions, coverage test, reduction) runs
on device.

Device pipeline per core:
  1. camera basis R from eye (look_at, mirrored op-for-op from the reference)
  2. v_cam = (verts - eye) @ R^T via PE matmuls  (verts pre-gathered per
     face-corner: 1280 faces x 4 corners (a,b,c,a) = 5120 columns)
  3. perspective: x_ndc = x / (z*tan + eps)
  4. edge coefficients per face-edge: e(x,y) = A*x + B*y + C
  5. rasterize 10 face-tiles of 128 faces x 4096 pixels: e-planes via
     broadcast adds, min3/max3 sign tests, visibility fold, bf16 indicator
     accumulation
  6. count = ones^T @ acc (PE partition reduction), silhouette = count > 0
"""

import sys

if "/opt/trn_rl_repo" not in sys.path:
    sys.path.insert(0, "/opt/trn_rl_repo")

import numpy as np

import concourse.bass as bass
import concourse.bacc as bacc
import concourse.tile as tile
from concourse import mybir
from concourse.bass_utils import run_bass_kernel_spmd

F32 = mybir.dt.float32
BF16 = mybir.dt.bfloat16
I32 = mybir.dt.int32
OP = mybir.AluOpType
AF = mybir.ActivationFunctionType

B, V, NF, IMG = 8, 642, 1280, 64
NPIX = IMG * IMG          # 4096
NTILE = NF // 128         # 10 face tiles
NCOL = NF * 4             # 5120 gathered corners (a, b, c, a)
EPS = 1e-8
# tan(deg2rad(15)) in float32, matching jnp.tan(jnp.deg2rad(float32(15)))
TAN_T = float(np.tan(np.deg2rad(np.float32(15.0)).astype(np.float32)))
BIG = 1e30


def _normalize3(nc, pool, v, name):
    """v [1,3] f32 -> v / (||v|| + 1e-8), mirroring the reference formula."""
    sq = pool.tile([1, 3], F32, tag=f"{name}_sq")
    nc.vector.tensor_tensor(sq[:], v[:], v[:], OP.mult)
    s = pool.tile([1, 1], F32, tag=f"{name}_s")
    nc.vector.tensor_reduce(s[:], sq[:], mybir.AxisListType.X, OP.add)
    n = pool.tile([1, 1], F32, tag=f"{name}_n")
    nc.scalar.activation(n[:], s[:], AF.Sqrt)
    # Newton refine sqrt: n1 = 0.5*(n + s/n)
    rn = pool.tile([1, 1], F32, tag=f"{name}_rn")
    nc.vector.reciprocal(rn[:], n[:])
    t = pool.tile([1, 1], F32, tag=f"{name}_t")
    nc.vector.tensor_tensor(t[:], s[:], rn[:], OP.mult)
    t2 = pool.tile([1, 1], F32, tag=f"{name}_t2")
    nc.vector.tensor_tensor(t2[:], n[:], t[:], OP.add)
    n1 = pool.tile([1, 1], F32, tag=f"{name}_n1")
    nc.vector.tensor_scalar(n1[:], t2[:], 0.5, None, OP.mult)
    d = pool.tile([1, 1], F32, tag=f"{name}_d")
    nc.vector.tensor_scalar(d[:], n1[:], EPS, None, OP.add)
    r = pool.tile([1, 1], F32, tag=f"{name}_r")
    nc.vector.reciprocal(r[:], d[:])
    # Newton refine recip: r1 = r*(2 - d*r)
    u = pool.tile([1, 1], F32, tag=f"{name}_u")
    nc.vector.tensor_tensor(u[:], d[:], r[:], OP.mult)
    u2 = pool.tile([1, 1], F32, tag=f"{name}_u2")
    nc.vector.tensor_scalar(u2[:], u[:], -1.0, 2.0, OP.mult, OP.add)
    r1 = pool.tile([1, 1], F32, tag=f"{name}_r1")
    nc.vector.tensor_tensor(r1[:], r[:], u2[:], OP.mult)
    out = pool.tile([1, 3], F32, tag=f"{name}_out")
    nc.vector.tensor_scalar(out[:], v[:], r1[:], None, OP.mult)
    return out


def _cross3(nc, pool, a, b, name):
    """cross(a, b) for [1,3] tiles via duplicated [1,6] buffers."""
    a2 = pool.tile([1, 6], F32, tag=f"{name}_a2")
    nc.vector.tensor_copy(a2[:, 0:3], a[:])
    nc.vector.tensor_copy(a2[:, 3:6], a[:])
    b2 = pool.tile([1, 6], F32, tag=f"{name}_b2")
    nc.vector.tensor_copy(b2[:, 0:3], b[:])
    nc.vector.tensor_copy(b2[:, 3:6], b[:])
    m1 = pool.tile([1, 3], F32, tag=f"{name}_m1")
    nc.vector.tensor_tensor(m1[:], a2[:, 1:4], b2[:, 2:5], OP.mult)
    m2 = pool.tile([1, 3], F32, tag=f"{name}_m2")
    nc.vector.tensor_tensor(m2[:], a2[:, 2:5], b2[:, 1:4], OP.mult)
    out = pool.tile([1, 3], F32, tag=f"{name}_out")
    nc.vector.tensor_tensor(out[:], m1[:], m2[:], OP.subtract)
    return out


def build_kernel(ctx, tc):
    nc = tc.nc
    vgt_d = nc.dram_tensor("vgt", [3, NCOL], F32, kind="ExternalInput")
    eye_d = nc.dram_tensor("eye", [3], F32, kind="ExternalInput")
    sil_d = nc.dram_tensor("sil", [NPIX], F32, kind="ExternalOutput")

    cpool = ctx.enter_context(tc.tile_pool(name="cam", bufs=1))
    ppool = ctx.enter_context(tc.tile_pool(name="proj", bufs=1))
    gpool = ctx.enter_context(tc.tile_pool(name="grid", bufs=1))

    # ---- camera basis (partition 0, tiny tiles) ----
    eyeR = cpool.tile([1, 3], F32)
    nc.sync.dma_start(eyeR[:], eye_d.ap())
    eT = cpool.tile([3, 1], F32)
    nc.sync.dma_start(eT[:], eye_d.ap())

    nege = cpool.tile([1, 3], F32)
    nc.vector.tensor_scalar(nege[:], eyeR[:], -1.0, None, OP.mult)
    z_ax = _normalize3(nc, cpool, nege, "nz")

    xr = cpool.tile([1, 3], F32)
    nc.vector.memset(xr[:], 0.0)
    nc.vector.tensor_copy(xr[:, 0:1], z_ax[:, 2:3])
    nc.vector.tensor_scalar(xr[:, 2:3], z_ax[:, 0:1], -1.0, None, OP.mult)
    x_ax = _normalize3(nc, cpool, xr, "nx")

    yr = _cross3(nc, cpool, z_ax, x_ax, "cy")
    y_ax = _normalize3(nc, cpool, yr, "ny")

    # RT[c, d] = R[d, c]; column d of RT = axis row d
    rt = cpool.tile([3, 3], F32)
    for d, axis in enumerate([x_ax, y_ax, z_ax]):
        nc.sync.dma_start(rt[:, d : d + 1], axis[:])

    # ---- projection of 5120 gathered corners ----
    vca = ppool.tile([128, 120], F32)  # [p, (chunk c, coord d)]
    with tc.tile_pool(name="vg", bufs=1) as vgp, \
         tc.tile_pool(name="pvc", bufs=1, space="PSUM") as psvc:
        vgt = vgp.tile([3, NCOL], F32)
        nc.sync.dma_start(vgt[:], vgt_d.ap())
        vme = vgp.tile([3, NCOL], F32)
        nc.vector.tensor_scalar(vme[:], vgt[:], eT[:], None, OP.subtract)
        vcp = psvc.tile([128, 120], F32)
        for c in range(40):
            nc.tensor.matmul(
                vcp[:, 3 * c : 3 * c + 3],
                vme[:, 128 * c : 128 * (c + 1)],
                rt[:],
                start=True,
                stop=True,
            )
        nc.vector.tensor_copy(vca[:], vcp[:])

    vcav = vca[:].rearrange("p (c d) -> p c d", d=3)
    vx, vy, vz = vcav[:, :, 0], vcav[:, :, 1], vcav[:, :, 2]

    dn = ppool.tile([128, 40], F32)
    nc.vector.tensor_scalar(dn[:], vz, TAN_T, EPS, OP.mult, OP.add)
    rc0 = ppool.tile([128, 40], F32)
    nc.vector.reciprocal(rc0[:], dn[:])
    t = ppool.tile([128, 40], F32)
    nc.vector.tensor_tensor(t[:], dn[:], rc0[:], OP.mult)
    t2 = ppool.tile([128, 40], F32)
    nc.vector.tensor_scalar(t2[:], t[:], -1.0, 2.0, OP.mult, OP.add)
    rc = ppool.tile([128, 40], F32)
    nc.vector.tensor_tensor(rc[:], rc0[:], t2[:], OP.mult)

    xn = ppool.tile([128, 40], F32)
    nc.vector.tensor_tensor(xn[:], vx, rc[:], OP.mult)
    yn = ppool.tile([128, 40], F32)
    nc.vector.tensor_tensor(yn[:], vy, rc[:], OP.mult)

    # ---- edge coefficients: e = A*x + B*y + C per (face, edge) ----
    xnv = xn[:].rearrange("p (ft k) -> p ft k", k=4)
    ynv = yn[:].rearrange("p (ft k) -> p ft k", k=4)
    A = ppool.tile([128, 30], F32)
    nc.vector.tensor_tensor(A[:].rearrange("p (ft k) -> p ft k", k=3),
                            ynv[:, :, 0:3], ynv[:, :, 1:4], OP.subtract)
    Bc = ppool.tile([128, 30], F32)
    nc.vector.tensor_tensor(Bc[:].rearrange("p (ft k) -> p ft k", k=3),
                            xnv[:, :, 1:4], xnv[:, :, 0:3], OP.subtract)
    p1 = ppool.tile([128, 30], F32)
    nc.vector.tensor_tensor(p1[:].rearrange("p (ft k) -> p ft k", k=3),
                            xnv[:, :, 0:3], ynv[:, :, 1:4], OP.mult)
    p2 = ppool.tile([128, 30], F32)
    nc.vector.tensor_tensor(p2[:].rearrange("p (ft k) -> p ft k", k=3),
                            ynv[:, :, 0:3], xnv[:, :, 1:4], OP.mult)
    Cc = ppool.tile([128, 30], F32)
    nc.vector.tensor_tensor(Cc[:], p1[:], p2[:], OP.subtract)

    # visibility: all three corner z > 0 (strict), as a 0/1 per-face scalar
    vz4 = vca[:].rearrange("p (ft k d) -> p ft k d", k=4, d=3)
    mz1 = ppool.tile([128, 10], F32)
    nc.vector.tensor_tensor(mz1[:], vz4[:, :, 0, 2], vz4[:, :, 1, 2], OP.min)
    mz = ppool.tile([128, 10], F32)
    nc.vector.tensor_tensor(mz[:], mz1[:], vz4[:, :, 2, 2], OP.min)
    vg = ppool.tile([128, 10], F32)
    nc.vector.tensor_scalar(vg[:], mz[:], 0.0, None, OP.is_gt)

    # ---- pixel grids ----
    it32 = gpool.tile([128, IMG], I32)
    nc.gpsimd.iota(it32[:], pattern=[[1, IMG]], base=0, channel_multiplier=0)
    itf = gpool.tile([128, IMG], F32)
    nc.vector.tensor_copy(itf[:], it32[:])
    xg = gpool.tile([128, IMG], F32)  # x_j = j/32 - 63/64 (exact)
    nc.vector.tensor_scalar(xg[:], itf[:], 1.0 / 32.0, -63.0 / 64.0, OP.mult, OP.add)
    yg = gpool.tile([128, IMG], F32)  # y_i = -x_i
    nc.vector.tensor_scalar(yg[:], xg[:], -1.0, None, OP.mult)
    ones_bf = gpool.tile([128, 1], BF16)
    nc.vector.memset(ones_bf[:], 1.0)

    # visibility weights for the counting matmuls: +vg and -vg in bf16
    vgp_bf = ppool.tile([128, 10], BF16)
    nc.vector.tensor_copy(vgp_bf[:], vg[:])
    vgn_bf = ppool.tile([128, 10], BF16)
    nc.vector.tensor_scalar(vgn_bf[:], vg[:], -1.0, None, OP.mult)

    # visible-face count V and threshold -2V (counting identity:
    # count = T/2 + V with T = sum vg*(sign(min3) - sign(max3)),
    # silhouette <=> count > 0 <=> T > -2V; all quantities are exact ints)
    vgs = ppool.tile([128, 1], F32)
    nc.vector.tensor_reduce(vgs[:], vg[:], mybir.AxisListType.X, OP.add)
    vgs_bf = ppool.tile([128, 1], BF16)
    nc.vector.tensor_copy(vgs_bf[:], vgs[:])
    thr = gpool.tile([1, 1], F32)
    with tc.tile_pool(name="pv", bufs=1, space="PSUM") as psv:
        vtot = psv.tile([1, 1], F32)
        nc.tensor.matmul(vtot[:], ones_bf[:], vgs_bf[:], start=True, stop=True)
        nc.vector.tensor_scalar(thr[:], vtot[:], -2.0, None, OP.mult)

    # ---- rasterization ----
    # Per face-tile: e-planes for 3 edges in one fused broadcast add (DVE,
    # f32), sign via ScalarE (ACT Sign -> bf16), min/max chains in bf16 (2x
    # DVE), per-face +/-vis-weighted counting on the PE into PSUM.
    HALF = NPIX // 2            # 2048 pixels per half (i in [0,32) / [32,64))
    upool = ctx.enter_context(tc.tile_pool(name="uw", bufs=4))
    epool = ctx.enter_context(tc.tile_pool(name="e3", bufs=2))
    spool = ctx.enter_context(tc.tile_pool(name="s3", bufs=2))
    mpool = ctx.enter_context(tc.tile_pool(name="mm", bufs=6))
    pscnt = ctx.enter_context(tc.tile_pool(name="pcnt", bufs=1, space="PSUM"))
    cnt = pscnt.tile([1, NPIX], F32, tag="cnt")
    region_n = [0] * (NPIX // 512)
    TOTAL_MM = NTILE * 2 * 4  # matmuls per 512-region over the whole loop

    def count_mm(lhsT, plane, pix_off):
        # accumulate lhsT^T @ plane into cnt[pix_off : pix_off+2048]
        for c in range(4):
            off = pix_off + 512 * c
            r = off // 512
            nc.tensor.matmul(
                cnt[:, off : off + 512], lhsT[:], plane[:, 512 * c : 512 * (c + 1)],
                start=(region_n[r] == 0), stop=(region_n[r] == TOTAL_MM // 4 - 1),
            )
            region_n[r] += 1

    for ft in range(NTILE):
        u3 = upool.tile([128, 3 * IMG], F32, tag="u3")
        w3 = upool.tile([128, 3 * IMG], F32, tag="w3")
        for k in range(3):
            col = 3 * ft + k
            nc.vector.tensor_scalar(u3[:, IMG * k : IMG * (k + 1)], xg[:],
                                    A[:, col : col + 1], Cc[:, col : col + 1],
                                    OP.mult, OP.add)
            nc.vector.tensor_scalar(w3[:, IMG * k : IMG * (k + 1)], yg[:],
                                    Bc[:, col : col + 1], None, OP.mult)
        u3r = u3[:].rearrange("p (k j) -> p k j", k=3)
        w3r = w3[:].rearrange("p (k i) -> p k i", k=3)
        for h in range(2):
            e3 = epool.tile([128, 3 * HALF], F32, tag="e3")
            nc.vector.tensor_tensor(
                e3[:].rearrange("p (k i j) -> p k i j", k=3, i=IMG // 2),
                w3r[:, :, 32 * h : 32 * (h + 1)].unsqueeze(3)
                    .broadcast_to([128, 3, 32, IMG]),
                u3r.unsqueeze(2).broadcast_to([128, 3, 32, IMG]),
                OP.add,
            )
            s3 = spool.tile([128, 3 * HALF], BF16, tag="s3")
            nc.scalar.activation(s3[:], e3[:], AF.Sign)
            s3r = s3[:].rearrange("p (k x) -> p k x", k=3)
            sm1 = mpool.tile([128, HALF], BF16, tag="mm")
            nc.vector.tensor_tensor(sm1[:], s3r[:, 0], s3r[:, 1], OP.min)
            smin = mpool.tile([128, HALF], BF16, tag="mm")
            nc.vector.tensor_tensor(smin[:], sm1[:], s3r[:, 2], OP.min)
            sM1 = mpool.tile([128, HALF], BF16, tag="mm")
            nc.vector.tensor_tensor(sM1[:], s3r[:, 0], s3r[:, 1], OP.max)
            smax = mpool.tile([128, HALF], BF16, tag="mm")
            nc.vector.tensor_tensor(smax[:], sM1[:], s3r[:, 2], OP.max)
            count_mm(vgp_bf[:, ft : ft + 1], smin, HALF * h)
            count_mm(vgn_bf[:, ft : ft + 1], smax, HALF * h)

    # ---- threshold compare and output ----
    silb = gpool.tile([1, NPIX], F32)
    nc.vector.tensor_tensor(silb[:], cnt[:], thr[:].broadcast_to([1, NPIX]),
                            OP.is_gt)
    nc.sync.dma_start(sil_d.ap(), silb[:])


_NC = None


def _get_program():
    global _NC
    if _NC is None:
        nc = bacc.Bacc(
            "TRN2",
            target_bir_lowering=False,
            debug=False,
            enable_asserts=False,
            num_devices=B,
        )
        from contextlib import ExitStack

        with tile.TileContext(nc) as tc:
            with ExitStack() as ctx:
                build_kernel(ctx, tc)
        nc.compile()
        _NC = nc
    return _NC


def _host_layout(vertices, faces):
    """Pure indexing: gather per-face-corner vertices, layout [3, 5120] where
    column n = ft*512 + k*128 + p holds corner k of face ft*128+p."""
    faces4 = np.concatenate([faces, faces[:, :1]], axis=1)  # [1280, 4]
    vidx = faces4.reshape(NTILE, 128, 4).transpose(0, 2, 1).reshape(-1)  # [5120]
    out = []
    for b in range(B):
        vg = vertices[b][vidx]  # [5120, 3]
        out.append(np.ascontiguousarray(vg.T.astype(np.float32)))
    return out


def kernel(vertices, viewpoints, faces, img_size):
    vertices = np.asarray(vertices, dtype=np.float32)
    viewpoints = np.asarray(viewpoints, dtype=np.float32)
    faces = np.asarray(faces, dtype=np.int32)
    assert int(img_size) == IMG and vertices.shape == (B, V, 3)

    nc = _get_program()
    vgts = _host_layout(vertices, faces)
    in_maps = [
        {"vgt": vgts[b], "eye": np.ascontiguousarray(viewpoints[b])}
        for b in range(B)
    ]
    res = run_bass_kernel_spmd(nc, in_maps, core_ids=list(range(B)))
    sil = np.stack([res.results[b]["sil"] for b in range(B)])  # [8, 4096]
    return sil.reshape(B, 1, IMG, IMG).astype(np.float32)


if __name__ == "__main__":
    # quick self-exercise with random data
    rng = np.random.default_rng(0)
    verts = rng.standard_normal((B, V, 3), dtype=np.float32) * 0.5
    vps = rng.standard_normal((B, 3), dtype=np.float32)
    fcs = rng.integers(0, V, (NF, 3), dtype=np.int32)
    out = kernel(verts, vps, fcs, IMG)
    print(out.shape, out.sum())


# revision 16
# speedup vs baseline: 2.0380x; 1.2026x over previous
"""Trainium2 Bass kernel for nn_Mesh_Renderer: silhouette rasterizer.

Strategy: data-parallel over batch. Core b renders batch b's 64x64 silhouette
from 1280 triangles. Host-side work is layout only: slice per batch, gather
vertices[faces] (pure indexing, no arithmetic), transpose. All math (camera
transform, perspective divide, edge functions, coverage test, reduction) runs
on device.

Device pipeline per core:
  1. camera basis R from eye (look_at, mirrored op-for-op from the reference)
  2. v_cam = (verts - eye) @ R^T via PE matmuls  (verts pre-gathered per
     face-corner: 1280 faces x 4 corners (a,b,c,a) = 5120 columns)
  3. perspective: x_ndc = x / (z*tan + eps)
  4. edge coefficients per face-edge: e(x,y) = A*x + B*y + C
  5. rasterize 10 face-tiles of 128 faces x 4096 pixels: e-planes via fused
     broadcast adds (DVE edge 0, GPSIMD edges 1-2), per-edge sign on ScalarE
     (ACT Sign -> bf16), min3/max3 chains in bf16 (2x DVE)
  6. counting on the PE: cnt += (+vis)^T @ sign(min3) + (-vis)^T @ sign(max3)
     accumulated in PSUM; silhouette = cnt > -2 * n_visible (exact integer
     identity for "any face covers pixel", equality-at-zero included)
"""

import sys

if "/opt/trn_rl_repo" not in sys.path:
    sys.path.insert(0, "/opt/trn_rl_repo")

import numpy as np

import concourse.bacc as bacc
import concourse.tile as tile
from concourse import mybir
from concourse.bass_utils import run_bass_kernel_spmd

F32 = mybir.dt.float32
BF16 = mybir.dt.bfloat16
I32 = mybir.dt.int32
OP = mybir.AluOpType
AF = mybir.ActivationFunctionType

B, V, NF, IMG = 8, 642, 1280, 64
NPIX = IMG * IMG          # 4096
NTILE = NF // 128         # 10 face tiles
NCOL = NF * 4             # 5120 gathered corners (a, b, c, a)
EPS = 1e-8
# tan(deg2rad(15)) in float32, matching jnp.tan(jnp.deg2rad(float32(15)))
TAN_T = float(np.tan(np.deg2rad(np.float32(15.0)).astype(np.float32)))


def _normalize3(nc, pool, v, name):
    """v [1,3] f32 -> v / (||v|| + 1e-8), mirroring the reference formula."""
    sq = pool.tile([1, 3], F32, tag=f"{name}_sq")
    nc.vector.tensor_tensor(sq[:], v[:], v[:], OP.mult)
    s = pool.tile([1, 1], F32, tag=f"{name}_s")
    nc.vector.tensor_reduce(s[:], sq[:], mybir.AxisListType.X, OP.add)
    n = pool.tile([1, 1], F32, tag=f"{name}_n")
    nc.scalar.activation(n[:], s[:], AF.Sqrt)
    # Newton refine sqrt: n1 = 0.5*(n + s/n)
    rn = pool.tile([1, 1], F32, tag=f"{name}_rn")
    nc.vector.reciprocal(rn[:], n[:])
    t = pool.tile([1, 1], F32, tag=f"{name}_t")
    nc.vector.tensor_tensor(t[:], s[:], rn[:], OP.mult)
    t2 = pool.tile([1, 1], F32, tag=f"{name}_t2")
    nc.vector.tensor_tensor(t2[:], n[:], t[:], OP.add)
    n1 = pool.tile([1, 1], F32, tag=f"{name}_n1")
    nc.vector.tensor_scalar(n1[:], t2[:], 0.5, None, OP.mult)
    d = pool.tile([1, 1], F32, tag=f"{name}_d")
    nc.vector.tensor_scalar(d[:], n1[:], EPS, None, OP.add)
    r = pool.tile([1, 1], F32, tag=f"{name}_r")
    nc.vector.reciprocal(r[:], d[:])
    # Newton refine recip: r1 = r*(2 - d*r)
    u = pool.tile([1, 1], F32, tag=f"{name}_u")
    nc.vector.tensor_tensor(u[:], d[:], r[:], OP.mult)
    u2 = pool.tile([1, 1], F32, tag=f"{name}_u2")
    nc.vector.tensor_scalar(u2[:], u[:], -1.0, 2.0, OP.mult, OP.add)
    r1 = pool.tile([1, 1], F32, tag=f"{name}_r1")
    nc.vector.tensor_tensor(r1[:], r[:], u2[:], OP.mult)
    out = pool.tile([1, 3], F32, tag=f"{name}_out")
    nc.vector.tensor_scalar(out[:], v[:], r1[:], None, OP.mult)
    return out


def _cross3(nc, pool, a, b, name):
    """cross(a, b) for [1,3] tiles via duplicated [1,6] buffers."""
    a2 = pool.tile([1, 6], F32, tag=f"{name}_a2")
    nc.vector.tensor_copy(a2[:, 0:3], a[:])
    nc.vector.tensor_copy(a2[:, 3:6], a[:])
    b2 = pool.tile([1, 6], F32, tag=f"{name}_b2")
    nc.vector.tensor_copy(b2[:, 0:3], b[:])
    nc.vector.tensor_copy(b2[:, 3:6], b[:])
    m1 = pool.tile([1, 3], F32, tag=f"{name}_m1")
    nc.vector.tensor_tensor(m1[:], a2[:, 1:4], b2[:, 2:5], OP.mult)
    m2 = pool.tile([1, 3], F32, tag=f"{name}_m2")
    nc.vector.tensor_tensor(m2[:], a2[:, 2:5], b2[:, 1:4], OP.mult)
    out = pool.tile([1, 3], F32, tag=f"{name}_out")
    nc.vector.tensor_tensor(out[:], m1[:], m2[:], OP.subtract)
    return out


def build_kernel(ctx, tc):
    nc = tc.nc
    vgt_d = nc.dram_tensor("vgt", [3, NCOL], F32, kind="ExternalInput")
    eye_d = nc.dram_tensor("eye", [3], F32, kind="ExternalInput")
    sil_d = nc.dram_tensor("sil", [NPIX], F32, kind="ExternalOutput")

    cpool = ctx.enter_context(tc.tile_pool(name="cam", bufs=1))
    ppool = ctx.enter_context(tc.tile_pool(name="proj", bufs=1))
    gpool = ctx.enter_context(tc.tile_pool(name="grid", bufs=1))

    # ---- camera basis (partition 0, tiny tiles) ----
    eyeR = cpool.tile([1, 3], F32)
    nc.sync.dma_start(eyeR[:], eye_d.ap())
    eT = cpool.tile([3, 1], F32)
    nc.sync.dma_start(eT[:], eye_d.ap())

    nege = cpool.tile([1, 3], F32)
    nc.vector.tensor_scalar(nege[:], eyeR[:], -1.0, None, OP.mult)
    z_ax = _normalize3(nc, cpool, nege, "nz")

    xr = cpool.tile([1, 3], F32)
    nc.vector.memset(xr[:], 0.0)
    nc.vector.tensor_copy(xr[:, 0:1], z_ax[:, 2:3])
    nc.vector.tensor_scalar(xr[:, 2:3], z_ax[:, 0:1], -1.0, None, OP.mult)
    x_ax = _normalize3(nc, cpool, xr, "nx")

    yr = _cross3(nc, cpool, z_ax, x_ax, "cy")
    y_ax = _normalize3(nc, cpool, yr, "ny")

    # RT[c, d] = R[d, c]; column d of RT = axis row d
    rt = cpool.tile([3, 3], F32)
    for d, axis in enumerate([x_ax, y_ax, z_ax]):
        nc.sync.dma_start(rt[:, d : d + 1], axis[:])

    # ---- projection of 5120 gathered corners ----
    vca = ppool.tile([128, 120], F32)  # [p, (chunk c, coord d)]
    with tc.tile_pool(name="vg", bufs=1) as vgp, \
         tc.tile_pool(name="pvc", bufs=1, space="PSUM") as psvc:
        vgt = vgp.tile([3, NCOL], F32)
        nc.sync.dma_start(vgt[:], vgt_d.ap())
        vme = vgp.tile([3, NCOL], F32)
        nc.vector.tensor_scalar(vme[:], vgt[:], eT[:], None, OP.subtract)
        vcp = psvc.tile([128, 120], F32)
        for c in range(40):
            nc.tensor.matmul(
                vcp[:, 3 * c : 3 * c + 3],
                vme[:, 128 * c : 128 * (c + 1)],
                rt[:],
                start=True,
                stop=True,
            )
        nc.vector.tensor_copy(vca[:], vcp[:])

    vcav = vca[:].rearrange("p (c d) -> p c d", d=3)
    vx, vy, vz = vcav[:, :, 0], vcav[:, :, 1], vcav[:, :, 2]

    dn = ppool.tile([128, 40], F32)
    nc.vector.tensor_scalar(dn[:], vz, TAN_T, EPS, OP.mult, OP.add)
    rc0 = ppool.tile([128, 40], F32)
    nc.vector.reciprocal(rc0[:], dn[:])
    t = ppool.tile([128, 40], F32)
    nc.vector.tensor_tensor(t[:], dn[:], rc0[:], OP.mult)
    t2 = ppool.tile([128, 40], F32)
    nc.vector.tensor_scalar(t2[:], t[:], -1.0, 2.0, OP.mult, OP.add)
    rc = ppool.tile([128, 40], F32)
    nc.vector.tensor_tensor(rc[:], rc0[:], t2[:], OP.mult)

    xn = ppool.tile([128, 40], F32)
    nc.vector.tensor_tensor(xn[:], vx, rc[:], OP.mult)
    yn = ppool.tile([128, 40], F32)
    nc.vector.tensor_tensor(yn[:], vy, rc[:], OP.mult)

    # ---- edge coefficients: e = A*x + B*y + C per (face, edge) ----
    xnv = xn[:].rearrange("p (ft k) -> p ft k", k=4)
    ynv = yn[:].rearrange("p (ft k) -> p ft k", k=4)
    A = ppool.tile([128, 30], F32)
    nc.vector.tensor_tensor(A[:].rearrange("p (ft k) -> p ft k", k=3),
                            ynv[:, :, 0:3], ynv[:, :, 1:4], OP.subtract)
    Bc = ppool.tile([128, 30], F32)
    nc.vector.tensor_tensor(Bc[:].rearrange("p (ft k) -> p ft k", k=3),
                            xnv[:, :, 1:4], xnv[:, :, 0:3], OP.subtract)
    p1 = ppool.tile([128, 30], F32)
    nc.vector.tensor_tensor(p1[:].rearrange("p (ft k) -> p ft k", k=3),
                            xnv[:, :, 0:3], ynv[:, :, 1:4], OP.mult)
    p2 = ppool.tile([128, 30], F32)
    nc.vector.tensor_tensor(p2[:].rearrange("p (ft k) -> p ft k", k=3),
                            ynv[:, :, 0:3], xnv[:, :, 1:4], OP.mult)
    Cc = ppool.tile([128, 30], F32)
    nc.vector.tensor_tensor(Cc[:], p1[:], p2[:], OP.subtract)

    # visibility: all three corner z > 0 (strict), as a 0/1 per-face scalar
    vz4 = vca[:].rearrange("p (ft k d) -> p ft k d", k=4, d=3)
    mz1 = ppool.tile([128, 10], F32)
    nc.vector.tensor_tensor(mz1[:], vz4[:, :, 0, 2], vz4[:, :, 1, 2], OP.min)
    mz = ppool.tile([128, 10], F32)
    nc.vector.tensor_tensor(mz[:], mz1[:], vz4[:, :, 2, 2], OP.min)
    vg = ppool.tile([128, 10], F32)
    nc.vector.tensor_scalar(vg[:], mz[:], 0.0, None, OP.is_gt)

    # ---- pixel grids ----
    it32 = gpool.tile([128, IMG], I32)
    nc.gpsimd.iota(it32[:], pattern=[[1, IMG]], base=0, channel_multiplier=0)
    itf = gpool.tile([128, IMG], F32)
    nc.vector.tensor_copy(itf[:], it32[:])
    xg = gpool.tile([128, IMG], F32)  # x_j = j/32 - 63/64 (exact)
    nc.vector.tensor_scalar(xg[:], itf[:], 1.0 / 32.0, -63.0 / 64.0, OP.mult, OP.add)
    yg = gpool.tile([128, IMG], F32)  # y_i = -x_i
    nc.vector.tensor_scalar(yg[:], xg[:], -1.0, None, OP.mult)
    ones_bf = gpool.tile([128, 1], BF16)
    nc.vector.memset(ones_bf[:], 1.0)

    # visibility weights for the counting matmuls: +vg and -vg in bf16
    vgp_bf = ppool.tile([128, 10], BF16)
    nc.vector.tensor_copy(vgp_bf[:], vg[:])
    vgn_bf = ppool.tile([128, 10], BF16)
    nc.vector.tensor_scalar(vgn_bf[:], vg[:], -1.0, None, OP.mult)

    # visible-face count V and threshold -2V (counting identity:
    # count = T/2 + V with T = sum vg*(sign(min3) - sign(max3)),
    # silhouette <=> count > 0 <=> T > -2V; all quantities are exact ints)
    vgs = ppool.tile([128, 1], F32)
    nc.vector.tensor_reduce(vgs[:], vg[:], mybir.AxisListType.X, OP.add)
    vgs_bf = ppool.tile([128, 1], BF16)
    nc.vector.tensor_copy(vgs_bf[:], vgs[:])
    thr = gpool.tile([1, 1], F32)
    with tc.tile_pool(name="pv", bufs=1, space="PSUM") as psv:
        vtot = psv.tile([1, 1], F32)
        nc.tensor.matmul(vtot[:], ones_bf[:], vgs_bf[:], start=True, stop=True)
        nc.vector.tensor_scalar(thr[:], vtot[:], -2.0, None, OP.mult)

    # ---- rasterization ----
    # Per face-tile: e-planes for 3 edges in one fused broadcast add (DVE,
    # f32), sign via ScalarE (ACT Sign -> bf16), min/max chains in bf16 (2x
    # DVE), per-face +/-vis-weighted counting on the PE into PSUM.
    HALF = NPIX // 2            # 2048 pixels per half (i in [0,32) / [32,64))
    upool = ctx.enter_context(tc.tile_pool(name="uw", bufs=4))
    epool = ctx.enter_context(tc.tile_pool(name="e3", bufs=2))
    spool = ctx.enter_context(tc.tile_pool(name="s3", bufs=2))
    mpool = ctx.enter_context(tc.tile_pool(name="mm", bufs=6))
    pscnt = ctx.enter_context(tc.tile_pool(name="pcnt", bufs=1, space="PSUM"))
    cnt = pscnt.tile([1, NPIX], F32, tag="cnt")
    region_n = [0] * (NPIX // 512)
    TOTAL_MM = NTILE * 2 * 4  # matmuls per 512-region over the whole loop

    def count_mm(lhsT, plane, pix_off):
        # accumulate lhsT^T @ plane into cnt[pix_off : pix_off+2048]
        for c in range(4):
            off = pix_off + 512 * c
            r = off // 512
            nc.tensor.matmul(
                cnt[:, off : off + 512], lhsT[:], plane[:, 512 * c : 512 * (c + 1)],
                start=(region_n[r] == 0), stop=(region_n[r] == TOTAL_MM // 4 - 1),
            )
            region_n[r] += 1

    for ft in range(NTILE):
        u3 = upool.tile([128, 3 * IMG], F32, tag="u3")
        w3 = upool.tile([128, 3 * IMG], F32, tag="w3")
        for k in range(3):
            col = 3 * ft + k
            nc.vector.tensor_scalar(u3[:, IMG * k : IMG * (k + 1)], xg[:],
                                    A[:, col : col + 1], Cc[:, col : col + 1],
                                    OP.mult, OP.add)
            nc.vector.tensor_scalar(w3[:, IMG * k : IMG * (k + 1)], yg[:],
                                    Bc[:, col : col + 1], None, OP.mult)
        u3r = u3[:].rearrange("p (k j) -> p k j", k=3)
        w3r = w3[:].rearrange("p (k i) -> p k i", k=3)
        for h in range(2):
            e3 = epool.tile([128, 3 * HALF], F32, tag="e3")
            e3v = e3[:].rearrange("p (k i j) -> p k i j", k=3, i=IMG // 2)
            # edges 0,1 on the vector engine; edge 2 on gpsimd in parallel
            nc.vector.tensor_tensor(
                e3v[:, 0:1],
                w3r[:, 0:1, 32 * h : 32 * (h + 1)].unsqueeze(3)
                    .broadcast_to([128, 1, 32, IMG]),
                u3r[:, 0:1].unsqueeze(2).broadcast_to([128, 1, 32, IMG]),
                OP.add,
            )
            nc.gpsimd.tensor_tensor(
                e3v[:, 1:3],
                w3r[:, 1:3, 32 * h : 32 * (h + 1)].unsqueeze(3)
                    .broadcast_to([128, 2, 32, IMG]),
                u3r[:, 1:3].unsqueeze(2).broadcast_to([128, 2, 32, IMG]),
                OP.add,
            )
            s3 = spool.tile([128, 3 * HALF], BF16, tag="s3")
            nc.scalar.activation(s3[:], e3[:], AF.Sign)
            s3r = s3[:].rearrange("p (k x) -> p k x", k=3)
            sm1 = mpool.tile([128, HALF], BF16, tag="mm")
            nc.vector.tensor_tensor(sm1[:], s3r[:, 0], s3r[:, 1], OP.min)
            smin = mpool.tile([128, HALF], BF16, tag="mm")
            nc.vector.tensor_tensor(smin[:], sm1[:], s3r[:, 2], OP.min)
            sM1 = mpool.tile([128, HALF], BF16, tag="mm")
            nc.vector.tensor_tensor(sM1[:], s3r[:, 0], s3r[:, 1], OP.max)
            smax = mpool.tile([128, HALF], BF16, tag="mm")
            nc.vector.tensor_tensor(smax[:], sM1[:], s3r[:, 2], OP.max)
            count_mm(vgp_bf[:, ft : ft + 1], smin, HALF * h)
            count_mm(vgn_bf[:, ft : ft + 1], smax, HALF * h)

    # ---- threshold compare and output ----
    silb = gpool.tile([1, NPIX], F32)
    nc.vector.tensor_tensor(silb[:], cnt[:], thr[:].broadcast_to([1, NPIX]),
                            OP.is_gt)
    nc.sync.dma_start(sil_d.ap(), silb[:])


_NC = None


def _get_program():
    global _NC
    if _NC is None:
        nc = bacc.Bacc(
            "TRN2",
            target_bir_lowering=False,
            debug=False,
            enable_asserts=False,
            num_devices=B,
        )
        from contextlib import ExitStack

        with tile.TileContext(nc) as tc:
            with ExitStack() as ctx:
                build_kernel(ctx, tc)
        nc.compile()
        _NC = nc
    return _NC


def _host_layout(vertices, faces):
    """Pure indexing: gather per-face-corner vertices, layout [3, 5120] where
    column n = ft*512 + k*128 + p holds corner k of face ft*128+p."""
    faces4 = np.concatenate([faces, faces[:, :1]], axis=1)  # [1280, 4]
    vidx = faces4.reshape(NTILE, 128, 4).transpose(0, 2, 1).reshape(-1)  # [5120]
    out = []
    for b in range(B):
        vg = vertices[b][vidx]  # [5120, 3]
        out.append(np.ascontiguousarray(vg.T.astype(np.float32)))
    return out


def kernel(vertices, viewpoints, faces, img_size):
    vertices = np.asarray(vertices, dtype=np.float32)
    viewpoints = np.asarray(viewpoints, dtype=np.float32)
    faces = np.asarray(faces, dtype=np.int32)
    assert int(img_size) == IMG and vertices.shape == (B, V, 3)

    nc = _get_program()
    vgts = _host_layout(vertices, faces)
    in_maps = [
        {"vgt": vgts[b], "eye": np.ascontiguousarray(viewpoints[b])}
        for b in range(B)
    ]
    res = run_bass_kernel_spmd(nc, in_maps, core_ids=list(range(B)))
    sil = np.stack([res.results[b]["sil"] for b in range(B)])  # [8, 4096]
    return sil.reshape(B, 1, IMG, IMG).astype(np.float32)


if __name__ == "__main__":
    # quick self-exercise with random data
    rng = np.random.default_rng(0)
    verts = rng.standard_normal((B, V, 3), dtype=np.float32) * 0.5
    vps = rng.standard_normal((B, 3), dtype=np.float32)
    fcs = rng.integers(0, V, (NF, 3), dtype=np.int32)
    out = kernel(verts, vps, fcs, IMG)
    print(out.shape, out.sum())
